# revision 2
# baseline (speedup 1.0000x reference)
"""ALiBi positional-embedding bias kernel for 8 TRN2 NeuronCores.

Reference math (B=8, H=8, L=1024, TOKEN_NUM=100):
    out[b,h,i,j] = ( tri[i,j] + slopes[h] * base[b,i,j] ) / 5
with tri = -inf on the strict upper triangle (0 elsewhere) and
    base[i,j] = kv[j] + eq[i,j]*(thc[i,j] + resp[j]) - oxth[j]*delta(i,j)
    kv[j]     = j + s2[j] + oxth[j]
    s2[j]     = (101-d[j]) if 101-d[j] > 50.5 else 0
    ox[j]     = 101-d[j] if resp[j]==1 else d[j];  oxth = ox if ox > 50.5 else 0
    eq[i,j]   = [d[i]==d[j]]
    cnt[i,j]  = #{j' <= j : d[j']==d[i]};  thc = cnt if cnt > 102.4 else 0
Since slopes > 0, folding -inf into base before the per-h scale is exact.

Sharding: data-parallel over batch, one batch row per core; slopes
replicated; each core emits its own [H, L, L] block independently.
"""

import numpy as np

import concourse.bacc as bacc
import concourse.mybir as mybir
import concourse.tile as tile
from concourse.bass_utils import run_bass_kernel_spmd

B, H, L = 8, 8, 1024
TN1 = 101.0  # TOKEN_NUM + 1
N_CORES = 8
FP32 = mybir.dt.float32
BF16 = mybir.dt.bfloat16
I32 = mybir.dt.int32
NEG_INF = float("-inf")

_CACHED_NC = None


def build_nc():
    nc = bacc.Bacc("TRN2", target_bir_lowering=False, debug=False,
                   num_devices=N_CORES)

    d_ext = nc.dram_tensor("diff", [1, L], FP32, kind="ExternalInput")
    r_ext = nc.dram_tensor("resp", [1, L], FP32, kind="ExternalInput")
    s_ext = nc.dram_tensor("slopes", [1, H], FP32, kind="ExternalInput")
    out_ext = nc.dram_tensor("out", [H, L, L], FP32, kind="ExternalOutput")

    AL = mybir.AluOpType

    with tile.TileContext(nc) as tc:
        with (
            tc.tile_pool(name="const", bufs=1) as cpool,
            tc.tile_pool(name="work", bufs=3) as wpool,
            tc.tile_pool(name="outp", bufs=6) as opool,
            tc.tile_pool(name="psum", bufs=2, space="PSUM") as ppool,
        ):
            # ---- preamble: rows, broadcasts, onehot, scan, kv ----
            db = cpool.tile([128, L], FP32)    # diff broadcast over partitions
            rb = cpool.tile([128, L], FP32)    # resp broadcast
            slv = cpool.tile([128, H], FP32)   # slopes broadcast
            nc.sync.dma_start(out=db[:], in_=d_ext[:].to_broadcast([128, L]))
            nc.sync.dma_start(out=rb[:], in_=r_ext[:].to_broadcast([128, L]))
            nc.sync.dma_start(out=slv[:], in_=s_ext[:].to_broadcast([128, H]))

            slv5 = cpool.tile([128, H], FP32)  # slopes / 5, per-partition scalars
            nc.vector.tensor_scalar_mul(slv5[:], slv[:], 0.2)

            iota_p_i = cpool.tile([128, 1], I32)
            nc.gpsimd.iota(iota_p_i[:], pattern=[[0, 1]], channel_multiplier=1)
            iota_p = cpool.tile([128, 1], FP32)
            nc.vector.tensor_copy(out=iota_p[:], in_=iota_p_i[:])

            jint = cpool.tile([128, L], I32)
            nc.gpsimd.iota(jint[:], pattern=[[1, L]], channel_multiplier=0)
            jbf = cpool.tile([128, L], FP32)
            nc.vector.tensor_copy(out=jbf[:], in_=jint[:])

            # onehot[v,j] = [d[j] == v]; whot = onehot * resp[j]
            onehot = cpool.tile([128, L], BF16)
            nc.vector.tensor_scalar(onehot[:], db[:], iota_p[:], None,
                                    op0=AL.is_equal)
            whot = cpool.tile([128, L], BF16)
            nc.vector.scalar_tensor_tensor(whot[:], db[:], iota_p[:], rb[:],
                                           op0=AL.is_equal, op1=AL.mult)
            # cumhot[v,j] = cumsum_j onehot[v,j]   (values <= L, small ints)
            cumhot = cpool.tile([128, L], BF16)
            nc.vector.tensor_tensor_scan(cumhot[:], onehot[:], onehot[:], 0.0,
                                         op0=AL.add, op1=AL.bypass)

            # s2 = (101-d)*[101-d > 50.5]
            de2 = cpool.tile([128, L], FP32)
            nc.vector.tensor_scalar(de2[:], db[:], -1.0, TN1,
                                    op0=AL.mult, op1=AL.add)
            s2b = cpool.tile([128, L], FP32)
            nc.vector.scalar_tensor_tensor(s2b[:], de2[:], 50.5, de2[:],
                                           op0=AL.is_gt, op1=AL.mult)
            # ox = d + r*(101 - 2d) = (d*-2+101)*r + d = de2p*r + d? Use:
            # u = (rb mult de2?) -> ox = d*(1-r) + (101-d)*r = d + r*(101-2d)
            rd = cpool.tile([128, L], FP32)
            nc.vector.tensor_mul(rd[:], rb[:], db[:])
            u1 = cpool.tile([128, L], FP32)  # d - 2*r*d
            nc.vector.scalar_tensor_tensor(u1[:], rd[:], -2.0, db[:],
                                           op0=AL.mult, op1=AL.add)
            oxb = cpool.tile([128, L], FP32)  # + 101*r
            nc.vector.scalar_tensor_tensor(oxb[:], rb[:], TN1, u1[:],
                                           op0=AL.mult, op1=AL.add)
            oxthb = cpool.tile([128, L], FP32)
            nc.vector.scalar_tensor_tensor(oxthb[:], oxb[:], 50.5, oxb[:],
                                           op0=AL.is_gt, op1=AL.mult)
            # kv = j + s2 + oxth (broadcast over partitions)
            kpart = cpool.tile([128, L], FP32)
            nc.vector.tensor_add(kpart[:], jbf[:], s2b[:])
            kvb = cpool.tile([128, L], FP32)
            nc.vector.tensor_add(kvb[:], kpart[:], oxthb[:])

            # exact bf16 hi/lo split of the kv row for PSUM accumulation
            kv_hi = cpool.tile([1, L], BF16)
            nc.vector.tensor_copy(out=kv_hi[:], in_=kvb[0:1, :])
            kv_hi_f = cpool.tile([1, L], FP32)
            nc.vector.tensor_copy(out=kv_hi_f[:], in_=kv_hi[:])
            kv_lo = cpool.tile([1, L], BF16)
            nc.vector.tensor_sub(kv_lo[:], kvb[0:1, :], kv_hi_f[:])

            ones_row = cpool.tile([1, 128], BF16)
            nc.vector.memset(ones_row[:], 1.0)

            # ---- main loop over 8 row-tiles ----
            for r in range(L // 128):
                r0 = r * 128
                oh_r = onehot[:, r0:r0 + 128]  # stationary [128v, 128i]
                base_t = wpool.tile([128, L], FP32, tag="base")
                for c in range(2):
                    c0 = c * 512
                    sl_c = slice(c0, c0 + 512)
                    p_eq = ppool.tile([128, 512], FP32, tag="eq")
                    p_cnt = ppool.tile([128, 512], FP32, tag="cnt")
                    p_c = ppool.tile([128, 512], FP32, tag="c")
                    nc.tensor.matmul(p_eq[:], oh_r, onehot[:, sl_c])
                    nc.tensor.matmul(p_cnt[:], oh_r, cumhot[:, sl_c])
                    nc.tensor.matmul(p_c[:], oh_r, whot[:, sl_c],
                                     start=True, stop=False)
                    nc.tensor.matmul(p_c[:], ones_row[:], kv_hi[:, sl_c],
                                     start=False, stop=False)
                    nc.tensor.matmul(p_c[:], ones_row[:], kv_lo[:, sl_c],
                                     start=False, stop=True)
                    # thc = cnt*[cnt>102.4]; s4 = thc*eq; base = s4 + (s5+kv)
                    g2 = wpool.tile([128, 512], FP32, tag="g2")
                    nc.vector.tensor_scalar(g2[:], p_cnt[:], L * 0.1, None,
                                            op0=AL.is_gt)
                    thc = wpool.tile([128, 512], FP32, tag="thc")
                    nc.vector.tensor_mul(thc[:], g2[:], p_cnt[:])
                    s4 = wpool.tile([128, 512], FP32, tag="s4")
                    nc.vector.tensor_mul(s4[:], thc[:], p_eq[:])
                    nc.vector.tensor_add(base_t[:, sl_c], s4[:], p_c[:])

                # diagonal fix: base[p, r0+p] -= oxth[r0+p]
                dsel = wpool.tile([128, 128], FP32, tag="dsel")
                nc.gpsimd.affine_select(
                    dsel[:], oxthb[:, r0:r0 + 128],
                    pattern=[[-1, 128]], compare_op=AL.is_equal, fill=0.0,
                    base=0, channel_multiplier=1,
                )
                nc.vector.tensor_sub(base_t[:, r0:r0 + 128],
                                     base_t[:, r0:r0 + 128], dsel[:])

                # causal mask: -inf where j > r0 + p  (iota = r0 + p - j >= 0 keeps)
                nc.gpsimd.affine_select(
                    base_t[:], base_t[:],
                    pattern=[[-1, L]], compare_op=AL.is_ge, fill=NEG_INF,
                    base=r0, channel_multiplier=1,
                )

                # 8 output planes: out[h] = slopes[h]/5 * baseM
                for h in range(H):
                    o_t = opool.tile([128, L], FP32, tag="otile")
                    if h % 2 == 0:
                        nc.scalar.activation(
                            o_t[:], base_t[:],
                            mybir.ActivationFunctionType.Copy,
                            bias=0.0, scale=slv5[:, h:h + 1],
                        )
                    else:
                        nc.vector.tensor_scalar_mul(o_t[:], base_t[:],
                                                    slv5[:, h:h + 1])
                    nc.sync.dma_start(out=out_ext[h, r0:r0 + 128, :],
                                      in_=o_t[:])

    nc.compile()
    return nc


def kernel(tensor=None, slopes=None, diff=None, response=None):
    global _CACHED_NC
    if _CACHED_NC is None:
        _CACHED_NC = build_nc()
    nc = _CACHED_NC

    slopes = np.asarray(slopes, dtype=np.float32).reshape(1, H)
    diff_f = np.asarray(diff, dtype=np.float32)
    resp_f = np.asarray(response, dtype=np.float32)

    in_maps = [
        {
            "diff": np.ascontiguousarray(diff_f[b:b + 1, :]),
            "resp": np.ascontiguousarray(resp_f[b:b + 1, :]),
            "slopes": slopes,
        }
        for b in range(B)
    ]
    res = run_bass_kernel_spmd(nc, in_maps, core_ids=list(range(N_CORES)))
    out = np.stack(
        [np.asarray(res.results[b]["out"]).reshape(H, L, L) for b in range(B)],
        axis=0,
    )
    return out.astype(np.float32, copy=False)
